# revision 1
# baseline (speedup 1.0000x reference)
"""Double-centering kernel for Trainium2 (Bass/Tile), 8-core data parallel.

Computes T = -0.5 * (D - row_mean - col_mean + glob_mean) for
D: [256, 512, 512] f32, sharding the batch dim across 8 NeuronCores
(32 matrices per core, no cross-core communication).

Per-core layout: PAIRS of [512, 512] matrices are viewed as one
[128, 4096] SBUF tile (matrix m in cols m*2048..; partition p holds its
rows 4p..4p+3), so every DMA is one fully contiguous 2 MiB transfer.

Per-pair dataflow (engine balance is the point — DMA is the roofline):
  SP:     2 MiB load -> in_t (HWDGE ring)
  GPSIMD: S2 = c01+c23, S = S2a+S2b per matrix (partial col sums)
          2 MiB store <- v (SWDGE, separate DMA path from loads)
  PE:     C0 = ones[128,128]^T @ S -> PSUM    (col sums bcast, per matrix)
  ACT:    v_c = -0.5*D_c (accum_out a_c = -0.5*rowsum_c), separate v tile
          Csc = C0/1024 (accum_out gsum = g/1024)  (= 0.5*col_mean)
  DVE:    rowterm = -(a+gsum)/512             (= 0.5*row_mean - 0.5*glob_mean)
          v_c = (v_c + rowterm_c) + Csc       (scalar_tensor_tensor, in place)

in_t is only read by S2 and the v-pass, so its slot recycles early and
the load pipeline stays deep; v carries the tail (stt -> store).
"""

from contextlib import ExitStack

import numpy as np

import concourse.bacc as bacc
import concourse.tile as tile
from concourse import mybir
from concourse.bass_utils import run_bass_kernel_spmd

N_CORES = 8
B = 256
N = 512
B_LOC = B // N_CORES  # 32 matrices per core
PAIR = 2
N_PAIRS = B_LOC // PAIR  # 16 DMA pairs per core
P = 128
CHUNKS = N // P  # 4
FREE = CHUNKS * N  # 2048 elems per partition per matrix
PFREE = PAIR * FREE  # 4096 per pair tile

_COMPILED = None
LAST_RESULTS = None  # BassKernelResults of the most recent run (for test harness)


def _build():
    nc = bacc.Bacc("TRN2", target_bir_lowering=False, debug=False)
    d_in = nc.dram_tensor("d_in", [N_PAIRS, P, PFREE], mybir.dt.float32,
                          kind="ExternalInput")
    t_out = nc.dram_tensor("t_out", [N_PAIRS, P, PFREE], mybir.dt.float32,
                           kind="ExternalOutput")
    f32 = mybir.dt.float32

    with tile.TileContext(nc) as tc, ExitStack() as ctx:
        singles = ctx.enter_context(tc.tile_pool(name="singles", bufs=1))
        in_pool = ctx.enter_context(tc.tile_pool(name="in", bufs=4))
        v_pool = ctx.enter_context(tc.tile_pool(name="v", bufs=3))
        s2_pool = ctx.enter_context(tc.tile_pool(name="s2", bufs=2))
        s_pool = ctx.enter_context(tc.tile_pool(name="s", bufs=2))
        csc_pool = ctx.enter_context(tc.tile_pool(name="csc", bufs=3))
        small = ctx.enter_context(tc.tile_pool(name="small", bufs=6))
        psum = ctx.enter_context(tc.tile_pool(name="psum", bufs=4, space="PSUM"))

        ones_kk = singles.tile([P, P], f32)
        nc.vector.memset(ones_kk[:], 1.0)

        for bp in range(N_PAIRS):
            in_t = in_pool.tile([P, PFREE], f32)
            nc.sync.dma_start(out=in_t[:], in_=d_in[bp])

            # Partial column sums per matrix, both matrices in one op pair:
            # view pair as [128, 2, 2048]; halves add -> S2 [128, 2, 1024].
            in3 = in_t[:].rearrange("p (m f) -> p m f", m=PAIR)
            s2 = s2_pool.tile([P, PAIR, 2 * N], f32)
            nc.gpsimd.tensor_add(out=s2[:], in0=in3[:, :, :2 * N],
                                 in1=in3[:, :, 2 * N:])
            s = s_pool.tile([P, PAIR, N], f32)
            nc.gpsimd.tensor_add(out=s[:], in0=s2[:, :, :N], in1=s2[:, :, N:])

            # Column sums broadcast to all 128 partitions via all-ones
            # matmul, one per matrix (N<=512 fp32 per PSUM bank).
            csc = csc_pool.tile([P, PAIR, N], f32)
            gsum = small.tile([P, PAIR], f32)
            v = v_pool.tile([P, PFREE], f32)
            a = small.tile([P, PAIR * CHUNKS], f32)
            for m in range(PAIR):
                c0 = psum.tile([P, N], f32)
                nc.tensor.matmul(out=c0[:], lhsT=ones_kk[:], rhs=s[:, m, :],
                                 start=True, stop=True)

                # v_c = -0.5 * D_c (ACT); a_c = -0.5 * rowsum_c.
                for c in range(CHUNKS):
                    sl = slice(m * FREE + c * N, m * FREE + (c + 1) * N)
                    k = m * CHUNKS + c
                    nc.scalar.activation(out=v[:, sl], in_=in_t[:, sl],
                                         func=mybir.ActivationFunctionType.Copy,
                                         bias=0.0, scale=-0.5,
                                         accum_out=a[:, k:k + 1])

                # Csc = 0.5*col_mean (SBUF); gsum = g/1024 per partition.
                nc.scalar.activation(out=csc[:, m, :], in_=c0[:],
                                     func=mybir.ActivationFunctionType.Copy,
                                     bias=0.0, scale=1.0 / 1024.0,
                                     accum_out=gsum[:, m:m + 1])

            # rowterm = 0.5*row_mean - 0.5*glob_mean = -(a + gsum)/512,
            # per matrix (gsum differs between the two matrices).
            rowterm = small.tile([P, PAIR * CHUNKS], f32)
            for m in range(PAIR):
                ksl = slice(m * CHUNKS, (m + 1) * CHUNKS)
                nc.vector.tensor_scalar(out=rowterm[:, ksl], in0=a[:, ksl],
                                        scalar1=gsum[:, m:m + 1],
                                        scalar2=-1.0 / 512.0,
                                        op0=mybir.AluOpType.add,
                                        op1=mybir.AluOpType.mult)

            # out_c = (v_c + rowterm_c) + Csc, fused and in place.
            for m in range(PAIR):
                for c in range(CHUNKS):
                    sl = slice(m * FREE + c * N, m * FREE + (c + 1) * N)
                    k = m * CHUNKS + c
                    nc.vector.scalar_tensor_tensor(out=v[:, sl],
                                                   in0=v[:, sl],
                                                   scalar=rowterm[:, k:k + 1],
                                                   in1=csc[:, m, :],
                                                   op0=mybir.AluOpType.add,
                                                   op1=mybir.AluOpType.add)

            nc.gpsimd.dma_start(out=t_out[bp], in_=v[:])

    nc.compile()
    return nc


def _get_nc():
    global _COMPILED
    if _COMPILED is None:
        _COMPILED = _build()
    return _COMPILED


def kernel(D: np.ndarray) -> np.ndarray:
    global LAST_RESULTS
    D = np.ascontiguousarray(np.asarray(D), dtype=np.float32)
    assert D.shape == (B, N, N), D.shape
    shards = D.reshape(N_CORES, N_PAIRS, PAIR, P, FREE)
    # pair tile layout: [128, 2*2048] with matrix m at cols m*2048..
    shards = shards.transpose(0, 1, 3, 2, 4).reshape(N_CORES, N_PAIRS, P, PFREE)
    nc = _get_nc()
    in_maps = [{"d_in": np.ascontiguousarray(shards[i])} for i in range(N_CORES)]
    res = run_bass_kernel_spmd(nc, in_maps, core_ids=list(range(N_CORES)))
    LAST_RESULTS = res
    out = np.stack([res.results[i]["t_out"] for i in range(N_CORES)])
    out = out.reshape(N_CORES, N_PAIRS, P, PAIR, FREE).transpose(0, 1, 3, 2, 4)
    return np.ascontiguousarray(out).reshape(B, N, N)



# revision 2
# speedup vs baseline: 1.8622x; 1.8622x over previous
"""Double-centering kernel for Trainium2 (Bass/Tile), 8-core data parallel.

Computes T = -0.5 * (D - row_mean - col_mean + glob_mean) for
D: [256, 512, 512] f32, sharding the batch dim across 8 NeuronCores
(32 matrices per core, no cross-core communication).

The kernel runs in fp16 end-to-end (HBM traffic halves vs f32; the
centering error stays ~1e-3 relative, far inside tolerance) using the
sequential-centering identity:

    csc0[j] = colsum[j] / 1024            (PE: ones/1024 matmuls, PSUM accum)
    w       = -0.5*D + csc0               (DVE stt, accum_out -> rowsum(w))
    T       = w - rowsum(w)/512           (ACT Identity-bias / DVE ts)

The second step's row-accumulator absorbs the global-mean term exactly,
so no separate global sum is needed.

Per-core layout: QUADS of four [512, 512] matrices live in one
[128, 8192] fp16 SBUF tile (matrix m at cols m*2048.., partition p holds
its rows 4p..4p+3), so every DMA is one contiguous 2 MiB transfer.

Engine balance per core (the point -- DMA ~98us is the roofline):
  SP:     8x 2 MiB loads (HWDGE)
  PE:     csc0 = (ones/1024)^T @ D chunks, 4-chunk PSUM accumulation
  ACT:    csc0 PSUM->SBUF fp16 copy; pass B Identity-bias on most chunks
  DVE:    pass A stt (w = -0.5*D + csc0, accum rowsums); a' = -a/512;
          pass B tensor_scalar on the remaining chunks
  GPSIMD: 8x 2 MiB stores (SWDGE, separate DMA path from loads)
"""

from contextlib import ExitStack

import numpy as np

import concourse.bacc as bacc
import concourse.tile as tile
from concourse import mybir
from concourse.bass_utils import run_bass_kernel_spmd

N_CORES = 8
B = 256
N = 512
B_LOC = B // N_CORES  # 32 matrices per core
QUAD = 4  # matrices per DMA tile
N_QUADS = B_LOC // QUAD  # 8 tiles per core
P = 128
CHUNKS = N // P  # 4 row-chunks per matrix
FREE = CHUNKS * N  # 2048 elems per partition per matrix
QFREE = QUAD * FREE  # 8192 per quad tile

# Pass B split: of the 16 (matrix, chunk) slices per quad, this many go to
# DVE tensor_scalar; the rest go to ACT Identity-bias. Tuned for balance.
PASS_B_DVE = 2

_COMPILED = None
LAST_RESULTS = None  # BassKernelResults of the most recent run (for test harness)


def _build():
    nc = bacc.Bacc("TRN2", target_bir_lowering=False, debug=False)
    f16 = mybir.dt.float16
    f32 = mybir.dt.float32
    d_in = nc.dram_tensor("d_in", [N_QUADS, P, QFREE], f16, kind="ExternalInput")
    t_out = nc.dram_tensor("t_out", [N_QUADS, P, QFREE], f16, kind="ExternalOutput")

    with tile.TileContext(nc) as tc, ExitStack() as ctx:
        singles = ctx.enter_context(tc.tile_pool(name="singles", bufs=1))
        in_pool = ctx.enter_context(tc.tile_pool(name="in", bufs=3))
        w_pool = ctx.enter_context(tc.tile_pool(name="w", bufs=3))
        csc_pool = ctx.enter_context(tc.tile_pool(name="csc", bufs=2))
        small = ctx.enter_context(tc.tile_pool(name="small", bufs=4))
        psum = ctx.enter_context(tc.tile_pool(name="psum", bufs=2, space="PSUM"))

        # All-ones/1024 weight (2^-10, exact in fp16): one matmul with this
        # lhsT broadcasts column sums/1024 of its rhs to all 128 partitions.
        ones_k = singles.tile([P, P], f16)
        nc.vector.memset(ones_k[:], 1.0 / 1024.0)

        for q in range(N_QUADS):
            in_t = in_pool.tile([P, QFREE], f16)
            nc.sync.dma_start(out=in_t[:], in_=d_in[q])

            # csc0 = colsum/1024 per matrix, accumulated over the 4 row
            # chunks into one PSUM bank per matrix (FD=512 f32 = 1 bank).
            pt = psum.tile([P, QUAD * N], f32)
            for m in range(QUAD):
                for c in range(CHUNKS):
                    k = m * CHUNKS + c
                    nc.tensor.matmul(
                        out=pt[:, m * N:(m + 1) * N],
                        lhsT=ones_k[:],
                        rhs=in_t[:, k * N:(k + 1) * N],
                        start=(c == 0),
                        stop=(c == CHUNKS - 1),
                    )

            # PSUM -> SBUF fp16, all 4 matrices in one ACT instruction.
            csc = csc_pool.tile([P, QUAD * N], f16)
            nc.scalar.activation(out=csc[:], in_=pt[:],
                                 func=mybir.ActivationFunctionType.Copy,
                                 bias=0.0, scale=1.0)

            # Pass A: w = -0.5*D + csc0 (col-centered, scaled);
            # accum a[:,k] = rowsum(w chunk) -- absorbs the global term.
            w = w_pool.tile([P, QFREE], f16)
            a = small.tile([P, QUAD * CHUNKS], f32)
            for m in range(QUAD):
                for c in range(CHUNKS):
                    k = m * CHUNKS + c
                    sl = slice(k * N, (k + 1) * N)
                    nc.vector.scalar_tensor_tensor(
                        out=w[:, sl], in0=in_t[:, sl], scalar=-0.5,
                        in1=csc[:, m * N:(m + 1) * N],
                        op0=mybir.AluOpType.mult, op1=mybir.AluOpType.add,
                        accum_out=a[:, k:k + 1],
                    )

            # a' = -a/512 = -(row mean of w)
            ap_t = small.tile([P, QUAD * CHUNKS], f32)
            nc.vector.tensor_scalar(out=ap_t[:], in0=a[:],
                                    scalar1=-1.0 / 512.0, scalar2=None,
                                    op0=mybir.AluOpType.mult)

            # Pass B: T = w + a'[p, k], in place, split ACT/DVE.
            for m in range(QUAD):
                for c in range(CHUNKS):
                    k = m * CHUNKS + c
                    sl = slice(k * N, (k + 1) * N)
                    if k < QUAD * CHUNKS - PASS_B_DVE:
                        nc.scalar.activation(
                            out=w[:, sl], in_=w[:, sl],
                            func=mybir.ActivationFunctionType.Identity,
                            bias=ap_t[:, k:k + 1], scale=1.0)
                    else:
                        nc.vector.tensor_scalar(
                            out=w[:, sl], in0=w[:, sl],
                            scalar1=ap_t[:, k:k + 1], scalar2=None,
                            op0=mybir.AluOpType.add)

            nc.gpsimd.dma_start(out=t_out[q], in_=w[:])

    nc.compile()
    return nc


def _get_nc():
    global _COMPILED
    if _COMPILED is None:
        _COMPILED = _build()
    return _COMPILED


def kernel(D: np.ndarray) -> np.ndarray:
    global LAST_RESULTS
    D = np.asarray(D)
    assert D.shape == (B, N, N), D.shape
    Dh = D.astype(np.float16)
    # quad tile layout: [128, 4*2048] with matrix m at cols m*2048..,
    # partition p holding rows 4p..4p+3 of each matrix.
    view = Dh.reshape(N_CORES, N_QUADS, QUAD, P, FREE)
    shards = view.transpose(0, 1, 3, 2, 4).reshape(N_CORES, N_QUADS, P, QFREE)
    nc = _get_nc()
    in_maps = [{"d_in": np.ascontiguousarray(shards[i])} for i in range(N_CORES)]
    res = run_bass_kernel_spmd(nc, in_maps, core_ids=list(range(N_CORES)))
    LAST_RESULTS = res
    out = np.stack([res.results[i]["t_out"] for i in range(N_CORES)])
    out = out.reshape(N_CORES, N_QUADS, P, QUAD, FREE).transpose(0, 1, 3, 2, 4)
    return np.ascontiguousarray(out).reshape(B, N, N).astype(np.float32)
